# revision 3
# baseline (speedup 1.0000x reference)
"""DirectedGraphLayer (GNN message passing) on 8 Trainium2 NeuronCores.

out = relu(x @ W_self + b_self + segment_sum(edge_val * (x@W)[edge_col], edge_row))

Strategy (node/row parallelism per sharding hint):
  - Partition destination nodes across 8 cores (6250 each, padded to 6400 = 50*128).
  - Each core computes x_trans = x @ [W | W_self] for its own node shard only
    (host pre-transposes x so the matmul needs no on-device transpose), then an
    AllGather replicates x_trans rows (node-major, B*FOUT = 128 floats = 512B per
    row) to every core.
  - Edges are sharded by destination and sorted into per-destination-tile lists
    on the host.  For each tile of 128 destinations the core dma_gathers the
    source rows (512B each, full DMA rate) and segment-sums them on the tensor
    engine: a one-hot selector matrix S (S[e, d] = val_e if dest(e)==d) is built
    in one DVE op per 128-edge chunk and out += S.T @ gathered accumulates in
    PSUM.  dma_gather indices are int16, so the 51200-row gather source is
    addressed as two 25600-row halves.
  - Finally out_tile = relu(psum + self_c) is written back; the host reassembles
    the (B, N, FOUT) output.
"""

import numpy as np

import concourse.bacc as bacc
import concourse.mybir as mybir
import concourse.tile as tile
from concourse.bass_utils import run_bass_kernel_spmd

NCORES = 8
FIN = 128
FOUT = 64
B = 2
GROUP = 2  # dest tiles per dma_gather pair


def _plan(N, edge_row, edge_col, edge_val):
    """Host-side edge partitioning. Returns static sizes + per-core arrays."""
    npc = -(-N // NCORES)            # nodes per core
    tiles = -(-npc // 128)
    if tiles % GROUP:
        tiles += GROUP - tiles % GROUP
    npc_pad = tiles * 128
    rows_all = NCORES * npc_pad
    split = rows_all // 2
    assert split <= 32767 and rows_all - split <= 32768

    core = edge_row // npc                   # destination core of each edge
    d_local = edge_row - core * npc
    t_of_e = d_local // 128
    slot = d_local % 128
    src = (edge_col // npc) * npc_pad + (edge_col % npc)   # row in AG output
    half = (src >= split).astype(np.int64)

    # stable sort by (core, tile, half)
    key = (core * tiles + t_of_e) * 2 + half
    order = np.argsort(key, kind="stable")
    key_s = key[order]
    nkeys = NCORES * tiles * 2
    counts = np.bincount(key_s, minlength=nkeys).reshape(NCORES, tiles, 2)

    # static padded sizes per (tile, half): max over cores, rounded to 128
    pad = counts.max(axis=0)                      # (tiles, 2)
    pad = ((pad + 127) // 128) * 128
    pad = np.maximum(pad, 128)                    # ensure >=1 chunk per tile
    pad_lo, pad_hi = pad[:, 0], pad[:, 1]

    per_core_edges = int((pad_lo + pad_hi).sum())  # padded edges per core
    nchunks = per_core_edges // 128

    # destination offset of each real edge inside its core's padded layout
    block_off = np.zeros((tiles, 2), dtype=np.int64)
    run = 0
    chunk_tile = []     # tile index of each chunk (global chunk order)
    chunk_first = []    # chunk is first of its tile's accumulation group
    chunk_last = []
    gathers = []        # (half, tile0, n_idx, chunk_base) per gather instr
    for g in range(tiles // GROUP):
        ts = range(g * GROUP, (g + 1) * GROUP)
        for h in (0, 1):
            n = int(sum((pad_lo if h == 0 else pad_hi)[t] for t in ts))
            gathers.append((h, g * GROUP, n, run // 128))
            for t in ts:
                block_off[t, h] = run
                cnt = int((pad_lo if h == 0 else pad_hi)[t])
                for c in range(cnt // 128):
                    chunk_tile.append(t)
                    chunk_first.append(h == 0 and c == 0)
                    chunk_last.append(h == 1 and c == cnt // 128 - 1)
                run += cnt
    assert run == per_core_edges and len(chunk_tile) == nchunks

    # hi chunks of tile t are never "first": pad_hi>=1 chunk and pad_lo>=1 chunk
    # so first is the first lo chunk and last is the last hi chunk of the tile.

    # scatter real edges into the padded per-core layout
    # rank of each sorted edge within its key block:
    first_of_key = np.zeros(nkeys + 1, dtype=np.int64)
    np.cumsum(np.bincount(key_s, minlength=nkeys), out=first_of_key[1:])
    rank = np.arange(len(key_s)) - first_of_key[key_s]
    c_s = core[order]
    t_s = t_of_e[order]
    h_s = half[order]
    pos = block_off[t_s, h_s] + rank       # position within core's padded edges

    idx_vals = np.zeros((NCORES, per_core_edges), dtype=np.int16)
    slot_arr = np.zeros((NCORES, per_core_edges), dtype=np.float32)
    val_arr = np.zeros((NCORES, per_core_edges), dtype=np.float32)
    src_s = src[order]
    src_rel = np.where(h_s == 1, src_s - split, src_s).astype(np.int16)
    idx_vals[c_s, pos] = src_rel
    slot_arr[c_s, pos] = slot[order].astype(np.float32)
    val_arr[c_s, pos] = edge_val[order]

    # slot/val tiles: [p, c] = value of edge c*128+p
    slot_t = slot_arr.reshape(NCORES, nchunks, 128).transpose(0, 2, 1).copy()
    val_t = val_arr.reshape(NCORES, nchunks, 128).transpose(0, 2, 1).copy()

    # idx tiles: per gather block, [p, s] = idx[s*16 + p%16], replicated to 128 rows
    s_total = per_core_edges // 16
    idx_t = np.zeros((NCORES, 128, s_total), dtype=np.int16)
    for (h, t0, n, cb) in gathers:
        if n == 0:
            continue
        lo = cb * 8            # column offset: 128 idx per chunk = 8 cols
        blk = idx_vals[:, cb * 128: cb * 128 + n]            # (NCORES, n)
        wrapped = blk.reshape(NCORES, n // 16, 16).transpose(0, 2, 1)  # (NC,16,S)
        idx_t[:, :, lo: lo + n // 16] = np.tile(wrapped, (1, 8, 1))

    return dict(
        npc=npc, tiles=tiles, npc_pad=npc_pad, rows_all=rows_all, split=split,
        nchunks=nchunks, s_total=s_total, gathers=gathers,
        chunk_tile=chunk_tile, chunk_first=chunk_first, chunk_last=chunk_last,
        idx_t=idx_t, slot_t=slot_t, val_t=val_t,
    )


def _build(plan):
    tiles = plan["tiles"]
    npc_pad = plan["npc_pad"]
    rows_all = plan["rows_all"]
    split = plan["split"]
    nchunks = plan["nchunks"]
    s_total = plan["s_total"]
    f32 = mybir.dt.float32

    nc = bacc.Bacc("TRN2", target_bir_lowering=False, num_devices=NCORES)
    xT_in = nc.dram_tensor("xT", [128, B * npc_pad], f32, kind="ExternalInput")
    wcat_in = nc.dram_tensor("wcat", [128, 128], f32, kind="ExternalInput")
    bias_in = nc.dram_tensor("bias", [128, FOUT], f32, kind="ExternalInput")
    iota_in = nc.dram_tensor("iota", [128, 128], f32, kind="ExternalInput")
    idx_in = nc.dram_tensor("idx", [128, s_total], mybir.dt.int16, kind="ExternalInput")
    slot_in = nc.dram_tensor("slot", [128, nchunks], f32, kind="ExternalInput")
    val_in = nc.dram_tensor("val", [128, nchunks], f32, kind="ExternalInput")
    out_d = nc.dram_tensor("out", [npc_pad, 128], f32, kind="ExternalOutput")

    with tile.TileContext(nc) as tc:
        with (
            tc.tile_pool(name="persist", bufs=1) as pp,
            tc.tile_pool(name="dram", bufs=1, space="DRAM") as dram,
        ):
            wcat = pp.tile([128, 128], f32)
            nc.sync.dma_start(wcat[:], wcat_in.ap())
            bias = pp.tile([128, FOUT], f32)
            nc.sync.dma_start(bias[:], bias_in.ap())
            iota = pp.tile([128, 128], f32)
            nc.sync.dma_start(iota[:], iota_in.ap())
            idx = pp.tile([128, s_total], mybir.dt.int16)
            nc.sync.dma_start(idx[:], idx_in.ap())
            slot = pp.tile([128, nchunks], f32)
            nc.sync.dma_start(slot[:], slot_in.ap())
            val = pp.tile([128, nchunks], f32)
            nc.sync.dma_start(val[:], val_in.ap())
            self_buf = pp.tile([128, npc_pad], f32)

            ag_in = dram.tile([npc_pad, 128], f32)
            ag_out = dram.tile([rows_all, 128], f32, addr_space="Shared")

            # ---- phase 1: x_trans/self for own shard ----
            with (
                tc.tile_pool(name="ph1", bufs=1) as p1,
                tc.tile_pool(name="mmps", bufs=4, space="PSUM") as mmps,
            ):
                xT = p1.tile([128, B * npc_pad], f32)
                nc.sync.dma_start(xT[:], xT_in.ap())
                stage = p1.tile([128, tiles, 128], f32)
                for cb in range(B * tiles):
                    b, t = divmod(cb, tiles)
                    mm = mmps.tile([128, 128], f32, tag="mm")
                    nc.tensor.matmul(
                        mm[:], xT[:, cb * 128:(cb + 1) * 128], wcat[:],
                        start=True, stop=True,
                    )
                    nc.vector.tensor_copy(
                        stage[:, t, b * FOUT:(b + 1) * FOUT], mm[:, 0:FOUT]
                    )
                    nc.vector.tensor_add(
                        self_buf[:, t * 128 + b * FOUT: t * 128 + (b + 1) * FOUT],
                        mm[:, FOUT:128], bias[:],
                    )
                nc.sync.dma_start(
                    ag_in[:].rearrange("(t p) c -> p t c", p=128), stage[:]
                )

            nc.gpsimd.collective_compute(
                "AllGather",
                mybir.AluOpType.bypass,
                replica_groups=[list(range(NCORES))],
                ins=[ag_in[:].opt()],
                outs=[ag_out[:].opt()],
            )

            # ---- phase 2: gather + segment-sum + combine ----
            with (
                tc.tile_pool(name="ph2", bufs=2) as p2,
                tc.tile_pool(name="sel", bufs=6) as selp,
                tc.tile_pool(name="accps", bufs=4, space="PSUM") as accps,
                tc.tile_pool(name="outp", bufs=3) as outp,
            ):
                src_lo = ag_out[0:split, :]
                src_hi = ag_out[split:rows_all, :]
                gi = 0
                ci = 0
                for g in range(tiles // GROUP):
                    (h0, t0, n_lo, cb_lo) = plan["gathers"][gi]
                    (h1, _, n_hi, cb_hi) = plan["gathers"][gi + 1]
                    gi += 2
                    cg = (n_lo + n_hi) // 128
                    gath = p2.tile([128, cg, 128], f32, tag="gath")
                    c_lo = n_lo // 128
                    if n_lo:
                        nc.gpsimd.dma_gather(
                            gath[:, 0:c_lo, :], src_lo,
                            idx[:, cb_lo * 8: cb_lo * 8 + n_lo // 16],
                            n_lo, n_lo, 128, elem_step=128, single_packet=False,
                        )
                    if n_hi:
                        nc.gpsimd.dma_gather(
                            gath[:, c_lo:cg, :], src_hi,
                            idx[:, cb_hi * 8: cb_hi * 8 + n_hi // 16],
                            n_hi, n_hi, 128, elem_step=128, single_packet=False,
                        )
                    # chunk order within the group: tile-major (all of t0, then t1)
                    psums = {}
                    local = list(range(ci, ci + cg))
                    by_tile = {}
                    for lc in local:
                        by_tile.setdefault(plan["chunk_tile"][lc], []).append(lc)
                    for t, lcs in by_tile.items():
                        ps = accps.tile([128, 128], f32, tag="acc")
                        psums[t] = ps
                        for lc in lcs:
                            c_in_g = lc - ci
                            sel = selp.tile([128, 128], f32, tag="sel")
                            nc.vector.tensor_scalar(
                                sel[:], iota[:],
                                slot[:, lc:lc + 1], val[:, lc:lc + 1],
                                mybir.AluOpType.is_equal, mybir.AluOpType.mult,
                            )
                            nc.tensor.matmul(
                                ps[:], sel[:], gath[:, c_in_g, :],
                                start=plan["chunk_first"][lc],
                                stop=plan["chunk_last"][lc],
                            )
                        # combine: relu(psum + self) -> out rows
                        tmp = outp.tile([128, 128], f32, tag="tmp")
                        nc.vector.tensor_add(
                            tmp[:], ps[:], self_buf[:, t * 128:(t + 1) * 128]
                        )
                        ot = outp.tile([128, 128], f32, tag="ot")
                        nc.scalar.activation(
                            ot[:], tmp[:], mybir.ActivationFunctionType.Relu
                        )
                        nc.sync.dma_start(out_d[t * 128:(t + 1) * 128, :], ot[:])
                    ci += cg
    nc.compile()
    return nc


def _prepare(x, W, W_self, b_self, edge_row, edge_col, edge_val):
    Bx, N, fin = x.shape
    assert Bx == B and fin == FIN and W.shape == (FIN, FOUT)
    plan = _plan(N, edge_row.astype(np.int64), edge_col.astype(np.int64),
                 edge_val.astype(np.float32))
    npc, npc_pad = plan["npc"], plan["npc_pad"]

    wcat = np.concatenate([W, W_self], axis=1).astype(np.float32)  # (128,128)
    bias = np.tile(b_self.astype(np.float32)[None, :], (128, 1))   # (128,64)
    iota = np.tile(np.arange(128, dtype=np.float32)[None, :], (128, 1))

    in_maps = []
    for k in range(NCORES):
        lo = k * npc
        hi = min(N, lo + npc)
        xs = np.zeros((B, npc_pad, FIN), dtype=np.float32)
        xs[:, : hi - lo] = x[:, lo:hi]
        xT = xs.transpose(2, 0, 1).reshape(FIN, B * npc_pad).copy()
        in_maps.append({
            "xT": xT, "wcat": wcat, "bias": bias, "iota": iota,
            "idx": plan["idx_t"][k], "slot": plan["slot_t"][k],
            "val": plan["val_t"][k],
        })

    def assemble(results):
        outs = []
        for k in range(NCORES):
            o = results[k]["out"]          # (npc_pad, 128)
            lo = k * npc
            hi = min(N, lo + npc)
            outs.append(o[: hi - lo].reshape(hi - lo, B, FOUT).transpose(1, 0, 2))
        return np.concatenate(outs, axis=1)

    nc = _build(plan)
    return nc, in_maps, assemble


def kernel(x, W, W_self, b_self, edge_row, edge_col, edge_val):
    nc, in_maps, assemble = _prepare(
        np.asarray(x), np.asarray(W), np.asarray(W_self), np.asarray(b_self),
        np.asarray(edge_row), np.asarray(edge_col), np.asarray(edge_val),
    )
    res = run_bass_kernel_spmd(nc, in_maps, core_ids=list(range(NCORES)))
    return assemble(res.results)
